# revision 13
# baseline (speedup 1.0000x reference)
"""Trainium2 Bass kernel for a decoder layer (DecoderAttention).

Math (reference):
    x   = tok_emb[target_tokens] + pos_emb[:S]                   # [B,S,H]
    x   = attn(x, x,   Wq_s, Wk_s, Wv_s, causal=True)            # self-attn
    x   = attn(x, enc, Wq_c, Wk_c, Wv_c, causal=False)           # cross-attn
    out = x @ Wout + bout                                        # [B,S,V]
with B=4, S=512, ENC=1024, H=1024, V=32000, single-head over full hidden dim.

Sharding: 8 cores = 4 batches x 2 ranks.  Core c owns batch c//2 with pair
rank r = c%2.  The attention stack is SPLIT across the pair: rank r owns
seq rows [r*256,(r+1)*256) (its q rows and its k/v source rows) and enc
rows [r*512,(r+1)*512) (its cross k2/v2 source rows).  Six pair
AllGathers (k, v, k2, v2, att2 in two halves - 7MB total per pair)
assemble the full tensors each side needs.  Projection order is
k,v,k2,v2,q so each gather is kicked 25-40us before its consumer; att2 is
gathered per query-half so the first half's exchange hides behind the
second half's softmax/AV, and phase C starts on half-0 chains (qc order
0,2,1,3) while half 1 lands.  Phase C (the dominant output projection)
keeps the vocab-half split: rank r computes out[:, r*16000:(r+1)*16000].

All matmuls run in fp16 (same 1 cycle/row PE rate as f32r, half the
LDWEIGHTS and DMA cost; e2e rel err ~1.3e-3 vs the 2e-2 gate).  The
softmax path (mask add, max, exp, row-sum) stays exact fp32.  The output
is stored fp16 and upcast on host.

Layout: per-phase tensors live in single big SBUF tiles [128, n, chunk]
so every DRAM<->SBUF move is ONE dma descriptor, with matmul operands
taken as 2D slices.  Queues: sync = inputs/weights + gather loads,
scalar = gather stages + Wout stream + output stores, gpsimd = Wout
prefetch + collective triggers, vector = PSUM drains.
"""

import numpy as np

import concourse.mybir as mybir
import concourse.tile as tile
from concourse import bacc, bass
from concourse.bass_utils import run_bass_kernel_spmd
from concourse.masks import make_identity

P = 128
B, S, ENC, H, V = 4, 512, 1024, 1024, 32000
HT = H // P            # 8 h-tiles of 128
SC = S // P            # 4 seq chunks of 128
EC = ENC // P          # 8 encoder chunks
SQ = S // 2            # 256 seq rows owned per rank
EQ = ENC // 2          # 512 enc rows owned per rank
VSH = V // 2           # 16000 vocab columns per core
NV = 500               # vocab tile: 32*500 = 16000, all >= 256 (full rate)
NVC = VSH // NV        # 32
NCORES = 8
RG = [[0, 1], [2, 3], [4, 5], [6, 7]]   # core pairs (same batch)
F32 = mybir.dt.float32
F16 = mybir.dt.float16
SCALE = 1.0 / np.sqrt(H)


def build_program(has_b_s=False, has_b_c=False, has_bout=False):
    """Trace + compile the single-core SPMD program. Returns nc."""
    nc = bacc.Bacc("TRN2", target_bir_lowering=False, debug=False,
                   num_devices=NCORES)

    MMDT = F16

    xq_d = nc.dram_tensor("xqT", [H, SQ], MMDT, kind="ExternalInput")
    encq_d = nc.dram_tensor("encqT", [H, EQ], MMDT, kind="ExternalInput")
    mask_d = nc.dram_tensor("mask", [2, P, S], F32, kind="ExternalInput")
    wqs_d = nc.dram_tensor("Wq_s", [H, H], MMDT, kind="ExternalInput")
    wks_d = nc.dram_tensor("Wk_s", [H, H], MMDT, kind="ExternalInput")
    wvs_d = nc.dram_tensor("Wv_s", [H, H], MMDT, kind="ExternalInput")
    wqc_d = nc.dram_tensor("Wq_c", [H, H], MMDT, kind="ExternalInput")
    wkc_d = nc.dram_tensor("Wk_c", [H, H], MMDT, kind="ExternalInput")
    wvc_d = nc.dram_tensor("Wv_c", [H, H], MMDT, kind="ExternalInput")
    # Wout pre-tiled on host: [p, vc, hi, j] = Wout[hi*128+p, vc*NV+j]
    wout_d = nc.dram_tensor("Wout", [P, NVC, HT, NV], MMDT,
                            kind="ExternalInput")
    out_d = nc.dram_tensor("out", [S, VSH], F16, kind="ExternalOutput")
    if has_b_s:
        bqs_d = nc.dram_tensor("bq_s", [H], F32, kind="ExternalInput")
        bks_d = nc.dram_tensor("bk_s", [H], F32, kind="ExternalInput")
        bvs_d = nc.dram_tensor("bv_s", [H], F32, kind="ExternalInput")
    if has_b_c:
        bqc_d = nc.dram_tensor("bq_c", [H], F32, kind="ExternalInput")
        bkc_d = nc.dram_tensor("bk_c", [H], F32, kind="ExternalInput")
        bvc_d = nc.dram_tensor("bv_c", [H], F32, kind="ExternalInput")
    if has_bout:
        bout_d = nc.dram_tensor("bout", [VSH], F32, kind="ExternalInput")

    Exp = mybir.ActivationFunctionType.Exp
    AX = mybir.AxisListType.X
    ADD = mybir.AluOpType.add
    BP = mybir.AluOpType.bypass

    def load_bias(pool, b_dram, name):
        t = pool.tile([P, HT], F32, name=name)
        nc.sync.dma_start(out=t[:, :], in_=b_dram[:].rearrange("(hi p) -> p hi", p=P))
        return t

    with tile.TileContext(nc) as tc:
        with tc.tile_pool(name="persist", bufs=1) as persist, \
             tc.tile_pool(name="stat", bufs=4) as stat, \
             tc.tile_pool(name="smx", bufs=1) as smx, \
             tc.tile_pool(name="psum", bufs=6, space="PSUM") as psum, \
             tc.tile_pool(name="psum_tp", bufs=2, space="PSUM") as psum_tp, \
             tc.tile_pool(name="dram", bufs=1, space="DRAM") as dram:

            # ---- Wout chunk prefetch: pinned first so its SBUF does not
            # overlap phase A/B pools (a shared slot would stall the DMA on
            # the pool-release semaphore until cross-attn finishes).
            NPRE = 2
            wpre = persist.tile([P, NPRE, HT, NV], MMDT, name="wpre")

            ident = persist.tile([P, P], MMDT, name="ident")
            make_identity(nc, ident[:, :])

            # AllGather bounce buffers (DRAM)
            kvin = dram.tile([P, 4096], MMDT, name="kvin")
            kvout = dram.tile([2, P, 4096], MMDT, name="kvout")
            kv2in = dram.tile([P, 8192], MMDT, name="kv2in")
            kv2out = dram.tile([2, P, 8192], MMDT, name="kv2out")
            ain = [dram.tile([P, HT, P], MMDT, name=f"ain{j}") for j in range(2)]
            aout = [dram.tile([2, P, HT, P], MMDT, name=f"aout{j}")
                    for j in range(2)]

            def allgather(in_t, out_t):
                nc.gpsimd.collective_compute(
                    "AllGather", BP, replica_groups=RG,
                    ins=[in_t[...].opt()], outs=[out_t[...].opt()])

            # att1T_own / att2T halves persist across phases
            att1T = persist.tile([P, HT, SQ], MMDT, name="att1T")
            att2T = [persist.tile([P, HT, 2, P], MMDT, name=f"att2T{j}")
                     for j in range(2)]

            def load_w(w_dram, wname, pool):
                """Weight [H, H] -> one tile [p, hi, o]; lhsT slices [:,hi,oc]."""
                t = pool.tile([P, HT, H], MMDT, tag="w", name=wname)
                nc.sync.dma_start(
                    out=t[:, :, :],
                    in_=w_dram[:, :].rearrange("(hi p) o -> p hi o", p=P))
                return t

            def proj(dst, w_t, rhs_of, nq, bias_t):
                """dst[:, ho, :nq] = (W.T @ rhs)[ho-chunk] (+ bias)."""
                for ho in range(HT):
                    ps = psum.tile([P, nq], F32, tag="acc")
                    for hi in range(HT):
                        nc.tensor.matmul(
                            out=ps[:, :],
                            lhsT=w_t[:, hi, ho * P:(ho + 1) * P],
                            rhs=rhs_of(hi),
                            start=(hi == 0), stop=(hi == HT - 1),
                        )
                    if bias_t is not None:
                        nc.vector.tensor_scalar_add(dst[:, ho, :], ps[:, :],
                                                    bias_t[:, ho:ho + 1])
                    else:
                        nc.vector.tensor_copy(out=dst[:, ho, :], in_=ps[:, :])

            def softmax_rows(p_sb, s_sb):
                """p_sb = exp(SCALE*(s_sb - rowmax)); returns 1/rowsum [128,1]."""
                mx = stat.tile([P, 1], F32, tag="mx")
                nm = stat.tile([P, 1], F32, tag="nm")
                rs = stat.tile([P, 1], F32, tag="rs")
                ri = stat.tile([P, 1], F32, tag="ri")
                nc.vector.reduce_max(out=mx[:, :], in_=s_sb, axis=AX)
                nc.vector.tensor_scalar_mul(nm[:, :], mx[:, :], -SCALE)
                nc.scalar.activation(p_sb, s_sb, Exp, bias=nm[:, :], scale=SCALE,
                                     accum_out=rs[:, :])
                nc.vector.reciprocal(out=ri[:, :], in_=rs[:, :])
                return ri

            # ---------------- Phase A/B: attention, pair-split ----------------
            with tc.tile_pool(name="phB", bufs=1) as pB, \
                 tc.tile_pool(name="wstr", bufs=3) as wpool:
                pA = tc.alloc_tile_pool(name="phA", bufs=1)

                wk = load_w(wks_d, "wk", wpool)
                xq = pA.tile([P, HT, SQ], MMDT, name="xq")
                nc.sync.dma_start(
                    out=xq[:, :, :],
                    in_=xq_d[:, :].rearrange("(hi p) s -> p hi s", p=P))
                wv = load_w(wvs_d, "wv", wpool)
                encq = pB.tile([P, HT, EQ], MMDT, name="encq")
                nc.sync.dma_start(
                    out=encq[:, :, :],
                    in_=encq_d[:, :].rearrange("(hi p) e -> p hi e", p=P))
                wkc = load_w(wkc_d, "wkc", wpool)
                wvc = load_w(wvc_d, "wvc", wpool)
                masks = pA.tile([P, 2, S], F32, name="masks")
                nc.sync.dma_start(out=masks[:, :, :],
                                  in_=mask_d[:, :, :].rearrange("m p s -> p m s"))
                wq = load_w(wqs_d, "wq", wpool)      # reuses a freed slot
                wqc = load_w(wqc_d, "wqc", wpool)    # reuses a freed slot

                bq = bk = bv = bq2 = bk2 = bv2 = None
                if has_b_s:
                    bq = load_bias(pA, bqs_d, "bqs")
                    bk = load_bias(pA, bks_d, "bks")
                    bv = load_bias(pA, bvs_d, "bvs")
                if has_b_c:
                    bq2 = load_bias(pA, bqc_d, "bqc")
                    bk2 = load_bias(pA, bkc_d, "bkc")
                    bv2 = load_bias(pA, bvc_d, "bvc")

                # --- own-row projections; kick AllGathers ASAP ---
                kTo = pA.tile([P, HT, SQ], MMDT, name="kTo")
                proj(kTo, wk, lambda hi: xq[:, hi, :], SQ, bk)
                nc.scalar.dma_start(
                    out=kvin[:, 0:2048],
                    in_=kTo[:, :, :].rearrange("p hi s -> p (hi s)"))

                vSo = pA.tile([P, 2, H], MMDT, name="vSo")
                for hh in range(2):
                    for j in range(2):
                        ps = psum.tile([P, 512], F32, tag="acc")
                        for hi in range(HT):
                            nc.tensor.matmul(
                                out=ps[:, :],
                                lhsT=xq[:, hi, j * P:(j + 1) * P],
                                rhs=wv[:, hi, hh * 512:(hh + 1) * 512],
                                start=(hi == 0), stop=(hi == HT - 1),
                            )
                        nc.vector.tensor_copy(
                            out=vSo[:, j, hh * 512:(hh + 1) * 512], in_=ps[:, :])
                nc.scalar.dma_start(
                    out=kvin[:, 2048:4096],
                    in_=vSo[:, :, :].rearrange("p j h -> p (j h)"))
                allgather(kvin, kvout)

                k2o = pB.tile([P, HT, EQ], MMDT, name="k2o")
                proj(k2o, wkc, lambda hi: encq[:, hi, :], EQ, bk2)
                nc.scalar.dma_start(
                    out=kv2in[:, 0:4096],
                    in_=k2o[:, :, :].rearrange("p hi e -> p (hi e)"))

                v2o = pB.tile([P, 4, H], MMDT, name="v2o")
                for hh in range(2):
                    for j in range(4):
                        ps = psum.tile([P, 512], F32, tag="acc")
                        for hi in range(HT):
                            nc.tensor.matmul(
                                out=ps[:, :],
                                lhsT=encq[:, hi, j * P:(j + 1) * P],
                                rhs=wvc[:, hi, hh * 512:(hh + 1) * 512],
                                start=(hi == 0), stop=(hi == HT - 1),
                            )
                        nc.vector.tensor_copy(
                            out=v2o[:, j, hh * 512:(hh + 1) * 512], in_=ps[:, :])
                nc.scalar.dma_start(
                    out=kv2in[:, 4096:8192],
                    in_=v2o[:, :, :].rearrange("p j h -> p (j h)"))
                allgather(kv2in, kv2out)
                for i in range(NPRE):
                    nc.gpsimd.dma_start(out=wpre[:, i, :, :],
                                        in_=wout_d[:, i, :, :])

                qT = pA.tile([P, HT, SQ], MMDT, name="qT")
                proj(qT, wq, lambda hi: xq[:, hi, :], SQ, bq)

                # --- self-attention scores/softmax/av over full k ---
                kTf = pA.tile([P, HT, 2, SQ], MMDT, name="kTf")
                for r_ in range(2):
                    nc.sync.dma_start(
                        out=kTf[:, :, r_, :],
                        in_=kvout[r_, :, 0:2048].rearrange(
                            "p (hi s) -> p hi s", hi=HT))
                pT = pA.tile([P, SC, SQ], MMDT, name="pT")
                for j in range(2):
                    sp = psum.tile([P, S], F32, tag="acc")
                    for hi in range(HT):
                        nc.tensor.matmul(
                            out=sp[:, :],
                            lhsT=qT[:, hi, j * P:(j + 1) * P],
                            rhs=kTf[:, hi, :, :],
                            start=(hi == 0), stop=(hi == HT - 1),
                        )
                    ssb = smx.tile([P, S], F32, tag="sm_s")
                    nc.vector.tensor_tensor(out=ssb[:, :], in0=sp[:, :],
                                            in1=masks[:, j, :], op=ADD)
                    p_sb = smx.tile([P, S], F32, tag="sm_p")
                    ri = softmax_rows(p_sb[:, :], ssb[:, :])
                    pn = smx.tile([P, S], MMDT, tag="sm_n")
                    nc.vector.tensor_scalar_mul(pn[:, :], p_sb[:, :], ri[:, :])
                    for kc in range(SC):
                        tp = psum_tp.tile([P, P], MMDT, tag="tp")
                        nc.tensor.transpose(tp[:, :], pn[:, kc * P:(kc + 1) * P],
                                            ident[:, :])
                        nc.vector.tensor_copy(out=pT[:, kc, j * P:(j + 1) * P],
                                              in_=tp[:, :])

                vSf = pA.tile([P, 2, 2, H], MMDT, name="vSf")
                for r_ in range(2):
                    nc.sync.dma_start(
                        out=vSf[:, r_, :, :],
                        in_=kvout[r_, :, 2048:4096].rearrange(
                            "p (j h) -> p j h", j=2))
                for ho in range(HT):
                    ps = psum.tile([P, SQ], F32, tag="acc")
                    for kc in range(SC):
                        nc.tensor.matmul(
                            out=ps[:, :],
                            lhsT=vSf[:, kc // 2, kc % 2, ho * P:(ho + 1) * P],
                            rhs=pT[:, kc, :],
                            start=(kc == 0), stop=(kc == SC - 1),
                        )
                    if bv is not None:
                        nc.vector.tensor_scalar_add(att1T[:, ho, :], ps[:, :],
                                                    bv[:, ho:ho + 1])
                    else:
                        nc.vector.tensor_copy(out=att1T[:, ho, :], in_=ps[:, :])

                pA.release()

                # --- cross-attention; att2 gathered per query-half ---
                q2T = pB.tile([P, HT, SQ], MMDT, name="q2T")
                proj(q2T, wqc, lambda hi: att1T[:, hi, :], SQ, bq2)

                k2f = pB.tile([P, HT, 2, EQ], MMDT, name="k2f")
                for r_ in range(2):
                    nc.sync.dma_start(
                        out=k2f[:, :, r_, :],
                        in_=kv2out[r_, :, 0:4096].rearrange(
                            "p (hi e) -> p hi e", hi=HT))
                v2f = pB.tile([P, 2, 4, H], MMDT, name="v2f")
                for r_ in range(2):
                    nc.sync.dma_start(
                        out=v2f[:, r_, :, :],
                        in_=kv2out[r_, :, 4096:8192].rearrange(
                            "p (j h) -> p j h", j=4))
                p2T = pB.tile([P, EC, SQ], MMDT, name="p2T")
                for j in range(2):
                    s2 = smx.tile([P, ENC], F32, tag="sm_s2")
                    for eh in range(2):
                        sp = psum.tile([P, EQ], F32, tag="acc")
                        for hi in range(HT):
                            nc.tensor.matmul(
                                out=sp[:, :],
                                lhsT=q2T[:, hi, j * P:(j + 1) * P],
                                rhs=k2f[:, hi, eh, :],
                                start=(hi == 0), stop=(hi == HT - 1),
                            )
                        nc.vector.tensor_copy(out=s2[:, eh * EQ:(eh + 1) * EQ],
                                              in_=sp[:, :])
                    p_sb = smx.tile([P, ENC], F32, tag="sm_p2")
                    ri = softmax_rows(p_sb[:, :], s2[:, :])
                    pn = smx.tile([P, ENC], MMDT, tag="sm_n2")
                    nc.vector.tensor_scalar_mul(pn[:, :], p_sb[:, :], ri[:, :])
                    for ec in range(EC):
                        tp = psum_tp.tile([P, P], MMDT, tag="tp")
                        nc.tensor.transpose(tp[:, :], pn[:, ec * P:(ec + 1) * P],
                                            ident[:, :])
                        nc.vector.tensor_copy(out=p2T[:, ec, j * P:(j + 1) * P],
                                              in_=tp[:, :])

                    # AV for this query-half only, then exchange it while the
                    # other half's softmax runs
                    a2o = pB.tile([P, HT, P], MMDT, tag="a2o", name=f"a2o{j}")
                    for ho in range(HT):
                        ps = psum.tile([P, P], F32, tag="acc")
                        for ec in range(EC):
                            nc.tensor.matmul(
                                out=ps[:, :],
                                lhsT=v2f[:, ec // 4, ec % 4, ho * P:(ho + 1) * P],
                                rhs=p2T[:, ec, j * P:(j + 1) * P],
                                start=(ec == 0), stop=(ec == EC - 1),
                            )
                        if bv2 is not None:
                            nc.vector.tensor_scalar_add(a2o[:, ho, :], ps[:, :],
                                                        bv2[:, ho:ho + 1])
                        else:
                            nc.vector.tensor_copy(out=a2o[:, ho, :], in_=ps[:, :])
                    nc.scalar.dma_start(out=ain[j][:, :, :], in_=a2o[:, :, :])
                    allgather(ain[j], aout[j])
                    nc.sync.dma_start(
                        out=att2T[j][:, :, :, :],
                        in_=aout[j][:, :, :, :].rearrange("r p hi s -> p hi r s"))

            # ---------------- Phase C: output projection ----------------
            GRP = 4
            with tc.tile_pool(name="phC_w", bufs=5) as pW, \
                 tc.tile_pool(name="phC_o", bufs=2) as pO:

                ones_t = None
                if has_bout:
                    ones_t = persist.tile([1, P], MMDT, name="ones")
                    nc.vector.memset(ones_t[:, :], 1.0)

                for g0 in range(0, NVC, GRP):
                    wts = []
                    for vc in range(g0, g0 + GRP):
                        if vc < NPRE:
                            wts.append(lambda hi, vc=vc: wpre[:, vc, hi, :])
                        else:
                            wtt = pW.tile([P, HT, NV], MMDT, tag="wt")
                            nc.scalar.dma_start(out=wtt[:, :, :],
                                                in_=wout_d[:, vc, :, :])
                            wts.append(lambda hi, t=wtt: t[:, hi, :])
                    for qc in (0, 2, 1, 3):
                        rr, jh = qc // 2, qc % 2
                        osb_q = pO.tile([P, GRP * NV], F16, tag=f"osb{qc}",
                                        name=f"osb{qc}_{g0}")
                        for gi in range(GRP):
                            vc = g0 + gi
                            ps = psum.tile([P, NV], F32, tag="acc")
                            for hi in range(HT):
                                last = (hi == HT - 1) and not has_bout
                                nc.tensor.matmul(
                                    out=ps[:, :],
                                    lhsT=att2T[jh][:, hi, rr, :],
                                    rhs=wts[gi](hi),
                                    start=(hi == 0), stop=last,
                                )
                            if has_bout:
                                bo = pW.tile([1, NV], MMDT, tag="bo")
                                nc.gpsimd.dma_start(
                                    out=bo[:, :],
                                    in_=bout_d[vc * NV:(vc + 1) * NV][None, :])
                                nc.tensor.matmul(
                                    out=ps[:, :], lhsT=ones_t[:, :], rhs=bo[:, :],
                                    start=False, stop=True,
                                )
                            nc.vector.tensor_copy(
                                out=osb_q[:, gi * NV:(gi + 1) * NV], in_=ps[:, :])
                        nc.scalar.dma_start(
                            out=out_d[qc * P:(qc + 1) * P,
                                      g0 * NV:(g0 + GRP) * NV],
                            in_=osb_q[:, :],
                        )
    nc.compile()
    return nc


def _host_prep(inputs):
    """Numpy-side sharding/layout prep. Returns (in_maps, flags)."""
    enc = np.asarray(inputs["encoder_outputs"], dtype=np.float32)
    tok = np.asarray(inputs["target_tokens"]).astype(np.int64)
    tok_emb = np.asarray(inputs["tok_emb"], dtype=np.float32)
    pos_emb = np.asarray(inputs["pos_emb"], dtype=np.float32)
    x0 = tok_emb[tok] + pos_emb[:S][None, :, :]          # [B,S,H]
    xT = np.ascontiguousarray(x0.transpose(0, 2, 1)).astype(np.float16)
    encT = np.ascontiguousarray(enc.transpose(0, 2, 1)).astype(np.float16)

    ws = {k: np.ascontiguousarray(np.asarray(inputs[k], dtype=np.float32)
                                  .astype(np.float16))
          for k in ("Wq_s", "Wk_s", "Wv_s", "Wq_c", "Wk_c", "Wv_c")}
    wout = np.asarray(inputs["Wout"], dtype=np.float32).astype(np.float16)
    bs = {k: np.asarray(inputs[k], dtype=np.float32)
          for k in ("bq_s", "bk_s", "bv_s", "bq_c", "bk_c", "bv_c", "bout")}
    has_b_s = any(np.any(bs[k]) for k in ("bq_s", "bk_s", "bv_s"))
    has_b_c = any(np.any(bs[k]) for k in ("bq_c", "bk_c", "bv_c"))
    has_bout = bool(np.any(bs["bout"]))

    # additive causal mask: mask[qc, i, j] = 0 if j <= qc*128+i else -1e9
    j = np.arange(S)[None, None, :]
    i_glob = np.arange(S).reshape(SC, P)[:, :, None]
    mask = np.where(j <= i_glob, 0.0, -1e9).astype(np.float32)

    in_maps = []
    for c in range(NCORES):
        b, r = c // 2, c % 2
        wsh = wout[:, r * VSH:(r + 1) * VSH]
        wr = np.ascontiguousarray(
            wsh.reshape(HT, P, NVC, NV).transpose(1, 2, 0, 3))
        m = {
            "xqT": np.ascontiguousarray(xT[b][:, r * SQ:(r + 1) * SQ]),
            "encqT": np.ascontiguousarray(encT[b][:, r * EQ:(r + 1) * EQ]),
            "mask": np.ascontiguousarray(mask[2 * r:2 * r + 2]),
            **ws,
            "Wout": wr,
        }
        if has_b_s:
            m.update({k: bs[k] for k in ("bq_s", "bk_s", "bv_s")})
        if has_b_c:
            m.update({k: bs[k] for k in ("bq_c", "bk_c", "bv_c")})
        if has_bout:
            m["bout"] = np.ascontiguousarray(bs["bout"][r * VSH:(r + 1) * VSH])
        in_maps.append(m)
    return in_maps, (has_b_s, has_b_c, has_bout)


def assemble_output(results):
    out = np.empty((B, S, V), dtype=np.float32)
    for c in range(NCORES):
        b, r = c // 2, c % 2
        out[b, :, r * VSH:(r + 1) * VSH] = results[c]["out"].astype(np.float32)
    return out


def kernel(**inputs):
    in_maps, (has_b_s, has_b_c, has_bout) = _host_prep(inputs)
    nc = build_program(has_b_s=has_b_s, has_b_c=has_b_c, has_bout=has_bout)
    res = run_bass_kernel_spmd(nc, in_maps, list(range(NCORES)))
    return assemble_output(res.results)


# revision 14
# speedup vs baseline: 1.0479x; 1.0479x over previous
"""Trainium2 Bass kernel for a decoder layer (DecoderAttention).

Math (reference):
    x   = tok_emb[target_tokens] + pos_emb[:S]                   # [B,S,H]
    x   = attn(x, x,   Wq_s, Wk_s, Wv_s, causal=True)            # self-attn
    x   = attn(x, enc, Wq_c, Wk_c, Wv_c, causal=False)           # cross-attn
    out = x @ Wout + bout                                        # [B,S,V]
with B=4, S=512, ENC=1024, H=1024, V=32000, single-head over full hidden dim.

Sharding: 8 cores = 4 batches x 2 ranks.  Core c owns batch c//2 with pair
rank r = c%2.  The attention stack is SPLIT across the pair: rank r owns
seq rows [r*256,(r+1)*256) (its q rows and its k/v source rows) and enc
rows [r*512,(r+1)*512) (its cross k2/v2 source rows).  Six pair
AllGathers (k, v, k2, v2, att2 in two halves - 7MB total per pair)
assemble the full tensors each side needs.  Projection order is
k,v,k2,v2,q so each gather is kicked 25-40us before its consumer; att2 is
gathered per query-half so the first half's exchange hides behind the
second half's softmax/AV, and phase C starts on half-0 chains (qc order
0,2,1,3) while half 1 lands.  Phase C (the dominant output projection)
keeps the vocab-half split: rank r computes out[:, r*16000:(r+1)*16000].

All matmuls run in fp16 (same 1 cycle/row PE rate as f32r, half the
LDWEIGHTS and DMA cost; e2e rel err ~1.3e-3 vs the 2e-2 gate).  The
softmax path (mask add, max, exp, row-sum) stays exact fp32.  The output
is stored fp16 and upcast on host.

Layout: per-phase tensors live in single big SBUF tiles [128, n, chunk]
so every DRAM<->SBUF move is ONE dma descriptor, with matmul operands
taken as 2D slices.  Queues: sync = inputs/weights + gather loads,
scalar = gather stages + Wout stream + output stores, gpsimd = Wout
prefetch + collective triggers, vector = PSUM drains.
"""

import numpy as np

import concourse.mybir as mybir
import concourse.tile as tile
from concourse import bacc, bass
from concourse.bass_utils import run_bass_kernel_spmd
from concourse.masks import make_identity

P = 128
B, S, ENC, H, V = 4, 512, 1024, 1024, 32000
HT = H // P            # 8 h-tiles of 128
SC = S // P            # 4 seq chunks of 128
EC = ENC // P          # 8 encoder chunks
SQ = S // 2            # 256 seq rows owned per rank
EQ = ENC // 2          # 512 enc rows owned per rank
VSH = V // 2           # 16000 vocab columns per core
NV = 500               # vocab tile: 32*500 = 16000, all >= 256 (full rate)
NVC = VSH // NV        # 32
NCORES = 8
RG = [[0, 1], [2, 3], [4, 5], [6, 7]]   # core pairs (same batch)
F32 = mybir.dt.float32
F16 = mybir.dt.float16
SCALE = 1.0 / np.sqrt(H)


def build_program(has_b_s=False, has_b_c=False, has_bout=False):
    """Trace + compile the single-core SPMD program. Returns nc."""
    nc = bacc.Bacc("TRN2", target_bir_lowering=False, debug=False,
                   num_devices=NCORES)

    MMDT = F16

    xq_d = nc.dram_tensor("xqT", [H, SQ], MMDT, kind="ExternalInput")
    encq_d = nc.dram_tensor("encqT", [H, EQ], MMDT, kind="ExternalInput")
    mask_d = nc.dram_tensor("mask", [2, P, S], F32, kind="ExternalInput")
    wqs_d = nc.dram_tensor("Wq_s", [H, H], MMDT, kind="ExternalInput")
    wks_d = nc.dram_tensor("Wk_s", [H, H], MMDT, kind="ExternalInput")
    wvs_d = nc.dram_tensor("Wv_s", [H, H], MMDT, kind="ExternalInput")
    wqc_d = nc.dram_tensor("Wq_c", [H, H], MMDT, kind="ExternalInput")
    wkc_d = nc.dram_tensor("Wk_c", [H, H], MMDT, kind="ExternalInput")
    wvc_d = nc.dram_tensor("Wv_c", [H, H], MMDT, kind="ExternalInput")
    # Wout pre-tiled on host: [p, vc, hi, j] = Wout[hi*128+p, vc*NV+j]
    wout_d = nc.dram_tensor("Wout", [P, NVC, HT, NV], MMDT,
                            kind="ExternalInput")
    out_d = nc.dram_tensor("out", [S, VSH], F16, kind="ExternalOutput")
    if has_b_s:
        bqs_d = nc.dram_tensor("bq_s", [H], F32, kind="ExternalInput")
        bks_d = nc.dram_tensor("bk_s", [H], F32, kind="ExternalInput")
        bvs_d = nc.dram_tensor("bv_s", [H], F32, kind="ExternalInput")
    if has_b_c:
        bqc_d = nc.dram_tensor("bq_c", [H], F32, kind="ExternalInput")
        bkc_d = nc.dram_tensor("bk_c", [H], F32, kind="ExternalInput")
        bvc_d = nc.dram_tensor("bv_c", [H], F32, kind="ExternalInput")
    if has_bout:
        bout_d = nc.dram_tensor("bout", [VSH], F32, kind="ExternalInput")

    Exp = mybir.ActivationFunctionType.Exp
    AX = mybir.AxisListType.X
    ADD = mybir.AluOpType.add
    BP = mybir.AluOpType.bypass

    def load_bias(pool, b_dram, name):
        t = pool.tile([P, HT], F32, name=name)
        nc.sync.dma_start(out=t[:, :], in_=b_dram[:].rearrange("(hi p) -> p hi", p=P))
        return t

    with tile.TileContext(nc) as tc:
        with tc.tile_pool(name="persist", bufs=1) as persist, \
             tc.tile_pool(name="stat", bufs=4) as stat, \
             tc.tile_pool(name="smx", bufs=1) as smx, \
             tc.tile_pool(name="psum", bufs=6, space="PSUM") as psum, \
             tc.tile_pool(name="psum_tp", bufs=2, space="PSUM") as psum_tp, \
             tc.tile_pool(name="dram", bufs=1, space="DRAM") as dram:

            # ---- Wout chunk prefetch: pinned first so its SBUF does not
            # overlap phase A/B pools (a shared slot would stall the DMA on
            # the pool-release semaphore until cross-attn finishes).
            NPRE = 2
            wpre = persist.tile([P, NPRE, HT, NV], MMDT, name="wpre")
            for i in range(NPRE):
                nc.gpsimd.dma_start(out=wpre[:, i, :, :], in_=wout_d[:, i, :, :])

            ident = persist.tile([P, P], MMDT, name="ident")
            make_identity(nc, ident[:, :])

            # AllGather bounce buffers (DRAM)
            kin = dram.tile([P, HT, SQ], MMDT, name="kin")
            kout = dram.tile([2, P, HT, SQ], MMDT, name="kout")
            vin = dram.tile([P, 2, H], MMDT, name="vin")
            vout = dram.tile([2, P, 2, H], MMDT, name="vout")
            k2in = dram.tile([P, HT, EQ], MMDT, name="k2in")
            k2out = dram.tile([2, P, HT, EQ], MMDT, name="k2out")
            v2in = dram.tile([P, 4, H], MMDT, name="v2in")
            v2out = dram.tile([2, P, 4, H], MMDT, name="v2out")
            ain = [dram.tile([P, HT, P], MMDT, name=f"ain{j}") for j in range(2)]
            aout = [dram.tile([2, P, HT, P], MMDT, name=f"aout{j}")
                    for j in range(2)]

            def allgather(in_t, out_t):
                nc.gpsimd.collective_compute(
                    "AllGather", BP, replica_groups=RG,
                    ins=[in_t[...].opt()], outs=[out_t[...].opt()])

            # att1T_own / att2T halves persist across phases
            att1T = persist.tile([P, HT, SQ], MMDT, name="att1T")
            att2T = [persist.tile([P, HT, 2, P], MMDT, name=f"att2T{j}")
                     for j in range(2)]

            def load_w(w_dram, wname, pool):
                """Weight [H, H] -> one tile [p, hi, o]; lhsT slices [:,hi,oc]."""
                t = pool.tile([P, HT, H], MMDT, tag="w", name=wname)
                nc.sync.dma_start(
                    out=t[:, :, :],
                    in_=w_dram[:, :].rearrange("(hi p) o -> p hi o", p=P))
                return t

            def proj(dst, w_t, rhs_of, nq, bias_t):
                """dst[:, ho, :nq] = (W.T @ rhs)[ho-chunk] (+ bias)."""
                for ho in range(HT):
                    ps = psum.tile([P, nq], F32, tag="acc")
                    for hi in range(HT):
                        nc.tensor.matmul(
                            out=ps[:, :],
                            lhsT=w_t[:, hi, ho * P:(ho + 1) * P],
                            rhs=rhs_of(hi),
                            start=(hi == 0), stop=(hi == HT - 1),
                        )
                    if bias_t is not None:
                        nc.vector.tensor_scalar_add(dst[:, ho, :], ps[:, :],
                                                    bias_t[:, ho:ho + 1])
                    else:
                        nc.vector.tensor_copy(out=dst[:, ho, :], in_=ps[:, :])

            def softmax_rows(p_sb, s_sb):
                """p_sb = exp(SCALE*(s_sb - rowmax)); returns 1/rowsum [128,1]."""
                mx = stat.tile([P, 1], F32, tag="mx")
                nm = stat.tile([P, 1], F32, tag="nm")
                rs = stat.tile([P, 1], F32, tag="rs")
                ri = stat.tile([P, 1], F32, tag="ri")
                nc.vector.reduce_max(out=mx[:, :], in_=s_sb, axis=AX)
                nc.vector.tensor_scalar_mul(nm[:, :], mx[:, :], -SCALE)
                nc.scalar.activation(p_sb, s_sb, Exp, bias=nm[:, :], scale=SCALE,
                                     accum_out=rs[:, :])
                nc.vector.reciprocal(out=ri[:, :], in_=rs[:, :])
                return ri

            # ---------------- Phase A/B: attention, pair-split ----------------
            with tc.tile_pool(name="phB", bufs=1) as pB, \
                 tc.tile_pool(name="wstr", bufs=3) as wpool:
                pA = tc.alloc_tile_pool(name="phA", bufs=1)

                xq = pA.tile([P, HT, SQ], MMDT, name="xq")
                nc.sync.dma_start(
                    out=xq[:, :, :],
                    in_=xq_d[:, :].rearrange("(hi p) s -> p hi s", p=P))
                wk = load_w(wks_d, "wk", wpool)
                wv = load_w(wvs_d, "wv", wpool)
                encq = pB.tile([P, HT, EQ], MMDT, name="encq")
                nc.sync.dma_start(
                    out=encq[:, :, :],
                    in_=encq_d[:, :].rearrange("(hi p) e -> p hi e", p=P))
                wkc = load_w(wkc_d, "wkc", wpool)
                wvc = load_w(wvc_d, "wvc", wpool)
                masks = pA.tile([P, 2, S], F32, name="masks")
                nc.sync.dma_start(out=masks[:, :, :],
                                  in_=mask_d[:, :, :].rearrange("m p s -> p m s"))
                wq = load_w(wqs_d, "wq", wpool)      # reuses a freed slot
                wqc = load_w(wqc_d, "wqc", wpool)    # reuses a freed slot

                bq = bk = bv = bq2 = bk2 = bv2 = None
                if has_b_s:
                    bq = load_bias(pA, bqs_d, "bqs")
                    bk = load_bias(pA, bks_d, "bks")
                    bv = load_bias(pA, bvs_d, "bvs")
                if has_b_c:
                    bq2 = load_bias(pA, bqc_d, "bqc")
                    bk2 = load_bias(pA, bkc_d, "bkc")
                    bv2 = load_bias(pA, bvc_d, "bvc")

                # --- own-row projections; kick AllGathers ASAP ---
                kTo = pA.tile([P, HT, SQ], MMDT, name="kTo")
                proj(kTo, wk, lambda hi: xq[:, hi, :], SQ, bk)
                nc.scalar.dma_start(out=kin[:, :, :], in_=kTo[:, :, :])
                allgather(kin, kout)

                vSo = pA.tile([P, 2, H], MMDT, name="vSo")
                for hh in range(2):
                    for j in range(2):
                        ps = psum.tile([P, 512], F32, tag="acc")
                        for hi in range(HT):
                            nc.tensor.matmul(
                                out=ps[:, :],
                                lhsT=xq[:, hi, j * P:(j + 1) * P],
                                rhs=wv[:, hi, hh * 512:(hh + 1) * 512],
                                start=(hi == 0), stop=(hi == HT - 1),
                            )
                        nc.vector.tensor_copy(
                            out=vSo[:, j, hh * 512:(hh + 1) * 512], in_=ps[:, :])
                nc.scalar.dma_start(out=vin[:, :, :], in_=vSo[:, :, :])
                allgather(vin, vout)

                k2o = pB.tile([P, HT, EQ], MMDT, name="k2o")
                proj(k2o, wkc, lambda hi: encq[:, hi, :], EQ, bk2)
                nc.scalar.dma_start(out=k2in[:, :, :], in_=k2o[:, :, :])
                allgather(k2in, k2out)

                v2o = pB.tile([P, 4, H], MMDT, name="v2o")
                for hh in range(2):
                    for j in range(4):
                        ps = psum.tile([P, 512], F32, tag="acc")
                        for hi in range(HT):
                            nc.tensor.matmul(
                                out=ps[:, :],
                                lhsT=encq[:, hi, j * P:(j + 1) * P],
                                rhs=wvc[:, hi, hh * 512:(hh + 1) * 512],
                                start=(hi == 0), stop=(hi == HT - 1),
                            )
                        nc.vector.tensor_copy(
                            out=v2o[:, j, hh * 512:(hh + 1) * 512], in_=ps[:, :])
                nc.scalar.dma_start(out=v2in[:, :, :], in_=v2o[:, :, :])
                allgather(v2in, v2out)

                qT = pA.tile([P, HT, SQ], MMDT, name="qT")
                proj(qT, wq, lambda hi: xq[:, hi, :], SQ, bq)

                # --- self-attention scores/softmax/av over full k ---
                kTf = pA.tile([P, HT, 2, SQ], MMDT, name="kTf")
                nc.sync.dma_start(
                    out=kTf[:, :, :, :],
                    in_=kout[:, :, :, :].rearrange("r p hi s -> p hi r s"))
                pT = pA.tile([P, SC, SQ], MMDT, name="pT")
                for j in range(2):
                    sp = psum.tile([P, S], F32, tag="acc")
                    for hi in range(HT):
                        nc.tensor.matmul(
                            out=sp[:, :],
                            lhsT=qT[:, hi, j * P:(j + 1) * P],
                            rhs=kTf[:, hi, :, :],
                            start=(hi == 0), stop=(hi == HT - 1),
                        )
                    ssb = smx.tile([P, S], F32, tag="sm_s")
                    nc.vector.tensor_tensor(out=ssb[:, :], in0=sp[:, :],
                                            in1=masks[:, j, :], op=ADD)
                    p_sb = smx.tile([P, S], F32, tag="sm_p")
                    ri = softmax_rows(p_sb[:, :], ssb[:, :])
                    pn = smx.tile([P, S], MMDT, tag="sm_n")
                    nc.vector.tensor_scalar_mul(pn[:, :], p_sb[:, :], ri[:, :])
                    for kc in range(SC):
                        tp = psum_tp.tile([P, P], MMDT, tag="tp")
                        nc.tensor.transpose(tp[:, :], pn[:, kc * P:(kc + 1) * P],
                                            ident[:, :])
                        nc.vector.tensor_copy(out=pT[:, kc, j * P:(j + 1) * P],
                                              in_=tp[:, :])

                vSf = pA.tile([P, 2, 2, H], MMDT, name="vSf")
                nc.sync.dma_start(
                    out=vSf[:, :, :, :],
                    in_=vout[:, :, :, :].rearrange("r p j h -> p r j h"))
                for ho in range(HT):
                    ps = psum.tile([P, SQ], F32, tag="acc")
                    for kc in range(SC):
                        nc.tensor.matmul(
                            out=ps[:, :],
                            lhsT=vSf[:, kc // 2, kc % 2, ho * P:(ho + 1) * P],
                            rhs=pT[:, kc, :],
                            start=(kc == 0), stop=(kc == SC - 1),
                        )
                    if bv is not None:
                        nc.vector.tensor_scalar_add(att1T[:, ho, :], ps[:, :],
                                                    bv[:, ho:ho + 1])
                    else:
                        nc.vector.tensor_copy(out=att1T[:, ho, :], in_=ps[:, :])

                pA.release()

                # --- cross-attention; att2 gathered per query-half ---
                q2T = pB.tile([P, HT, SQ], MMDT, name="q2T")
                proj(q2T, wqc, lambda hi: att1T[:, hi, :], SQ, bq2)

                k2f = pB.tile([P, HT, 2, EQ], MMDT, name="k2f")
                nc.sync.dma_start(
                    out=k2f[:, :, :, :],
                    in_=k2out[:, :, :, :].rearrange("r p hi e -> p hi r e"))
                v2f = pB.tile([P, 2, 4, H], MMDT, name="v2f")
                nc.sync.dma_start(
                    out=v2f[:, :, :, :],
                    in_=v2out[:, :, :, :].rearrange("r p j h -> p r j h"))
                p2T = pB.tile([P, EC, SQ], MMDT, name="p2T")
                for j in range(2):
                    s2 = smx.tile([P, ENC], F32, tag="sm_s2")
                    for eh in range(2):
                        sp = psum.tile([P, EQ], F32, tag="acc")
                        for hi in range(HT):
                            nc.tensor.matmul(
                                out=sp[:, :],
                                lhsT=q2T[:, hi, j * P:(j + 1) * P],
                                rhs=k2f[:, hi, eh, :],
                                start=(hi == 0), stop=(hi == HT - 1),
                            )
                        nc.vector.tensor_copy(out=s2[:, eh * EQ:(eh + 1) * EQ],
                                              in_=sp[:, :])
                    p_sb = smx.tile([P, ENC], F32, tag="sm_p2")
                    ri = softmax_rows(p_sb[:, :], s2[:, :])
                    pn = smx.tile([P, ENC], MMDT, tag="sm_n2")
                    nc.vector.tensor_scalar_mul(pn[:, :], p_sb[:, :], ri[:, :])
                    for ec in range(EC):
                        tp = psum_tp.tile([P, P], MMDT, tag="tp")
                        nc.tensor.transpose(tp[:, :], pn[:, ec * P:(ec + 1) * P],
                                            ident[:, :])
                        nc.vector.tensor_copy(out=p2T[:, ec, j * P:(j + 1) * P],
                                              in_=tp[:, :])

                    # AV for this query-half only, then exchange it while the
                    # other half's softmax runs
                    a2o = pB.tile([P, HT, P], MMDT, tag="a2o", name=f"a2o{j}")
                    for ho in range(HT):
                        ps = psum.tile([P, P], F32, tag="acc")
                        for ec in range(EC):
                            nc.tensor.matmul(
                                out=ps[:, :],
                                lhsT=v2f[:, ec // 4, ec % 4, ho * P:(ho + 1) * P],
                                rhs=p2T[:, ec, j * P:(j + 1) * P],
                                start=(ec == 0), stop=(ec == EC - 1),
                            )
                        if bv2 is not None:
                            nc.vector.tensor_scalar_add(a2o[:, ho, :], ps[:, :],
                                                        bv2[:, ho:ho + 1])
                        else:
                            nc.vector.tensor_copy(out=a2o[:, ho, :], in_=ps[:, :])
                    nc.scalar.dma_start(out=ain[j][:, :, :], in_=a2o[:, :, :])
                    allgather(ain[j], aout[j])
                    nc.sync.dma_start(
                        out=att2T[j][:, :, :, :],
                        in_=aout[j][:, :, :, :].rearrange("r p hi s -> p hi r s"))

            # ---------------- Phase C: output projection ----------------
            GRP = 4
            with tc.tile_pool(name="phC_w", bufs=5) as pW, \
                 tc.tile_pool(name="phC_o", bufs=2) as pO:

                ones_t = None
                if has_bout:
                    ones_t = persist.tile([1, P], MMDT, name="ones")
                    nc.vector.memset(ones_t[:, :], 1.0)

                osb = [None] * SC
                for vc in range(NVC):
                    g = vc % GRP
                    if vc < NPRE:
                        wt_of = lambda hi, vc=vc: wpre[:, vc, hi, :]
                    else:
                        wtt = pW.tile([P, HT, NV], MMDT, tag="wt")
                        nc.scalar.dma_start(out=wtt[:, :, :],
                                            in_=wout_d[:, vc, :, :])
                        wt_of = lambda hi, t=wtt: t[:, hi, :]
                    bo = None
                    if has_bout:
                        bo = pW.tile([1, NV], MMDT, tag="bo")
                        nc.gpsimd.dma_start(out=bo[:, :],
                                            in_=bout_d[vc * NV:(vc + 1) * NV][None, :])
                    for qc in (0, 2, 1, 3):
                        rr, jh = qc // 2, qc % 2
                        if g == 0:
                            osb[qc] = pO.tile([P, GRP * NV], F16, tag=f"osb{qc}",
                                              name=f"osb{qc}_{vc}")
                        ps = psum.tile([P, NV], F32, tag="acc")
                        for hi in range(HT):
                            last = (hi == HT - 1) and not has_bout
                            nc.tensor.matmul(
                                out=ps[:, :],
                                lhsT=att2T[jh][:, hi, rr, :],
                                rhs=wt_of(hi),
                                start=(hi == 0), stop=last,
                            )
                        if has_bout:
                            nc.tensor.matmul(
                                out=ps[:, :], lhsT=ones_t[:, :], rhs=bo[:, :],
                                start=False, stop=True,
                            )
                        nc.vector.tensor_copy(
                            out=osb[qc][:, g * NV:(g + 1) * NV], in_=ps[:, :])
                        if g == GRP - 1:
                            v0 = (vc - g) * NV
                            nc.scalar.dma_start(
                                out=out_d[qc * P:(qc + 1) * P, v0:v0 + GRP * NV],
                                in_=osb[qc][:, :],
                            )
    nc.compile()
    return nc


def _host_prep(inputs):
    """Numpy-side sharding/layout prep. Returns (in_maps, flags)."""
    enc = np.asarray(inputs["encoder_outputs"], dtype=np.float32)
    tok = np.asarray(inputs["target_tokens"]).astype(np.int64)
    tok_emb = np.asarray(inputs["tok_emb"], dtype=np.float32)
    pos_emb = np.asarray(inputs["pos_emb"], dtype=np.float32)
    x0 = tok_emb[tok] + pos_emb[:S][None, :, :]          # [B,S,H]
    xT = np.ascontiguousarray(x0.transpose(0, 2, 1)).astype(np.float16)
    encT = np.ascontiguousarray(enc.transpose(0, 2, 1)).astype(np.float16)

    ws = {k: np.ascontiguousarray(np.asarray(inputs[k], dtype=np.float32)
                                  .astype(np.float16))
          for k in ("Wq_s", "Wk_s", "Wv_s", "Wq_c", "Wk_c", "Wv_c")}
    wout = np.asarray(inputs["Wout"], dtype=np.float32).astype(np.float16)
    bs = {k: np.asarray(inputs[k], dtype=np.float32)
          for k in ("bq_s", "bk_s", "bv_s", "bq_c", "bk_c", "bv_c", "bout")}
    has_b_s = any(np.any(bs[k]) for k in ("bq_s", "bk_s", "bv_s"))
    has_b_c = any(np.any(bs[k]) for k in ("bq_c", "bk_c", "bv_c"))
    has_bout = bool(np.any(bs["bout"]))

    # additive causal mask: mask[qc, i, j] = 0 if j <= qc*128+i else -1e9
    j = np.arange(S)[None, None, :]
    i_glob = np.arange(S).reshape(SC, P)[:, :, None]
    mask = np.where(j <= i_glob, 0.0, -1e9).astype(np.float32)

    in_maps = []
    for c in range(NCORES):
        b, r = c // 2, c % 2
        wsh = wout[:, r * VSH:(r + 1) * VSH]
        wr = np.ascontiguousarray(
            wsh.reshape(HT, P, NVC, NV).transpose(1, 2, 0, 3))
        m = {
            "xqT": np.ascontiguousarray(xT[b][:, r * SQ:(r + 1) * SQ]),
            "encqT": np.ascontiguousarray(encT[b][:, r * EQ:(r + 1) * EQ]),
            "mask": np.ascontiguousarray(mask[2 * r:2 * r + 2]),
            **ws,
            "Wout": wr,
        }
        if has_b_s:
            m.update({k: bs[k] for k in ("bq_s", "bk_s", "bv_s")})
        if has_b_c:
            m.update({k: bs[k] for k in ("bq_c", "bk_c", "bv_c")})
        if has_bout:
            m["bout"] = np.ascontiguousarray(bs["bout"][r * VSH:(r + 1) * VSH])
        in_maps.append(m)
    return in_maps, (has_b_s, has_b_c, has_bout)


def assemble_output(results):
    out = np.empty((B, S, V), dtype=np.float32)
    for c in range(NCORES):
        b, r = c // 2, c % 2
        out[b, :, r * VSH:(r + 1) * VSH] = results[c]["out"].astype(np.float32)
    return out


def kernel(**inputs):
    in_maps, (has_b_s, has_b_c, has_bout) = _host_prep(inputs)
    nc = build_program(has_b_s=has_b_s, has_b_c=has_b_c, has_bout=has_bout)
    res = run_bass_kernel_spmd(nc, in_maps, list(range(NCORES)))
    return assemble_output(res.results)
